# revision 39
# baseline (speedup 1.0000x reference)
"""Trainium2 Bass kernel for AttentionalPlanarRemapping.

out[n,c,h,w] = sum_d softmax(atts[n,c,:])[d] * images[n,d,h,w]

Per-sample: W = softmax(atts[n]) [C,C]; out[n] = W @ images[n].reshape(C, H*W).

Sharding: data-parallel over N across 8 cores (4 samples per core).

Host preprocessing inside kernel(): both inputs are cast to fp16 and
pre-shuffled into the exact SBUF tile layout [P=128 partitions, KD, free]:
  x[n, p, kd, hw] = images[n, kd*128+p, hw]     (8KB contiguous/partition)
  a[n, p, kd, c]  = atts[n, c, kd*128+p]        (transposed: lhsT layout)
so every DMA descriptor is a 2-8KB contiguous run (the v2 trace showed atts
loads crawling at 38-110 GB/s on 1KB descriptors) and no on-device
transposition is needed. fp16 matches the PE's full-rate matmul path while
halving DMA traffic; |atts| < 6 so exp needs no max-subtraction. (fp8 was
measured offline: e4m3 on both operands gives rel_err 5.6e-2 > the 2e-2
budget, so DoubleRow is not available accuracy-wise.)

Per-core plan (v9 -- built from NTFF trace analysis over 9 iterations):
  - The early phase is chip-HBM-supply-bound: all 8 cores pull their
    sample-0 working set simultaneously, packets only start ~2.3us after
    the start barrier and the aggregate ramps 170 -> 430 GB/s, so
    sample-0's 1.5MB cannot fully land before ~14-15us no matter how the
    loads are scheduled. The design accepts that and removes every other
    stall instead.
  - The SDMA engines round-robin packets across ALL in-flight DMAs --
    per DMA, not per byte -- so (a) receipts cluster at the end of the
    busy window, (b) a chunk's bandwidth share equals its share of the
    in-flight DMA count, and (c) the only priority mechanism is limiting
    what is in flight. Sample 0 therefore rides as EXACTLY 8 triggers,
    with the earliest-needed x0 quarters (kd0, kd1) split into
    partition-halves for a double share; Tile rotates a pool of 8 HWDGE
    completion semaphores and a reused semaphore waits for its previous
    user, so every bulk trigger (the 9th onward) only enters flight once
    a sample-0 chunk has fully landed.
  - Loads ride the two HWDGE queues (sync=SP, scalar=ACT) in consumption
    order; gpsimd (SWDGE, ~131 GB/s ceiling) carries no loads. Both
    inputs are host-pre-shuffled so every descriptor is a 2-8KB
    contiguous run (1KB atts descriptors measured ~2x slower):
        scalar: a0(kd01), x0(kd1)x2, x0(kd3)
        sync:   x0(kd0)x2, a0(kd23), x0(kd2), x1, a1, x2, a2, a3, x3
    (x1..x3/a1..a3 are single full-tile triggers)
  - The PE HAM clock gate needs ~3.4us of sustained busy to reach 8/8 =
    2.4 GHz and re-throttles to 1.2 GHz after an idle window. The
    supply-paced kc0 phase (one x0 receipt per ~1us, each unlocking only
    ~0.55us of matmuls) would leave the PE ~50% idle right inside one HAM
    window -- measured as a guaranteed mid-stream re-throttle costing
    ~1.7us. So NWARM=80 dummy N=128 matmuls on a memset tile bridge the
    PE from the start barrier (~7.2us) all the way to data-ready
    (~13.4us): the real stream then runs gapless and warm to the end.
    (Cost of leftover dummies if data lands early: 53ns each.)
  - sample-0 exp runs as four 0.2us kc0-column slabs (gating kc0's
    LDWEIGHTS as early as possible -- Tile pipelines each block's
    LDWEIGHTS ahead of the previous block's matmuls, and a sem-blocked
    LDWEIGHTS head-of-line blocks the strict-FIFO PE queue) followed by
    four kc1-3 remainder chunks.
  - Main matmuls run on UNNORMALIZED E = exp(a): for each output block kc
    (128 rows of c) and each psum half ht, accumulate over kd:
        ps[ht][c128, 512] += E[kd-blk, kc-blk].T @ X[kd-blk, ht-half]
    Each [128,512] f32 psum tile is exactly one PSUM bank; 6 bufs rotate.
  - Softmax denominators ride along as tiny N=2 matmuls on the SAME loaded
    weights (sm[kc] += E.T @ ones), emitted before the two 512-col matmuls
    of each kd so the kd=3 stop retires early; r = 1/s via DVE reciprocal;
    evictions scale by r while casting to fp16 (ht0 on ACT for kc<3 else
    DVE, balancing ACT/DVE at ~5.1us each per sample; ht1 on DVE) -- at
    most one pending exp chunk can ever head-of-line block an ACT
    eviction, and DVE's evictions gate PSUM-bank reuse.
  - exp(n+1) is emitted one kd-chunk per kc block of compute(n).
  - Stores are deferred by one sample and alternate queues (n0,n2 ->
    gpsimd/SWDGE; n1 -> sync, idle once the loads drain). Sample 3 stores
    kc0/kc1 on sync and kc2 on gpsimd immediately after each eviction; the
    final kc3 block accumulates ht1 as two N=256 groups so the last
    eviction retires with a quarter tile, evicts fanned across ACT/DVE,
    and stores as two parallel halves on sync+scalar (1KB-line quarter
    stores measured only ~40 GB/s; a gpsimd half blocked SWDGE's
    end-of-kernel drain).

Measured exec_time_ns (same binary, shared devices, chip-power dependent):
v1 50-58.5us, v5 50.1us, v9 ~49.4-52us (median ~50.5); tail (last matmul
-> kernel end) 4.4us, PE stream at warm roofline throughout.
"""

import numpy as np
from contextlib import ExitStack

import concourse.bass as bass
import concourse.mybir as mybir
import concourse.tile as tile
from concourse import bacc
from concourse.bass_utils import run_bass_kernel_spmd

N, C, H, W = 32, 512, 32, 32
HW = H * W                      # 1024
NCORES = 8
NPC = N // NCORES               # 4 samples per core
P = 128
KC = C // P                     # 4 chunks over output channel c
KD = C // P                     # 4 chunks over contraction d
NT = 512                        # matmul moving free dim (one PSUM bank of f32)
NHT = HW // NT                  # 2
NWARM = 76                      # dummy N=128 matmuls bridging to data-ready
                                # (~7.2->13.4us): the supply-paced kc0 phase
                                # leaves the PE ~50% idle otherwise, which
                                # trips a HAM mid-stream re-throttle

F32 = mybir.dt.float32
F16 = mybir.dt.float16
AF = mybir.ActivationFunctionType


def build_nc():
    nc = bacc.Bacc("TRN2", target_bir_lowering=False, debug=False)

    # both inputs pre-shuffled on host into SBUF tile layout [P, KD, free]
    images = nc.dram_tensor("images", [NPC, P, KD, HW], F16, kind="ExternalInput").ap()
    attsT = nc.dram_tensor("attsT", [NPC, P, KD, C], F16, kind="ExternalInput").ap()
    out = nc.dram_tensor("out", [NPC, C, HW], F16, kind="ExternalOutput").ap()

    with ExitStack() as ctx:
        tc = ctx.enter_context(tile.TileContext(nc))

        const_pool = ctx.enter_context(tc.tile_pool(name="const", bufs=1))
        ones_f32 = const_pool.tile([P, 2], F32)
        ones = const_pool.tile([P, 2], F16)
        warm_x = const_pool.tile([P, P], F16)

        a_pool = ctx.enter_context(tc.tile_pool(name="a", bufs=1))
        e_pool = ctx.enter_context(tc.tile_pool(name="e", bufs=1))
        x_pool = ctx.enter_context(tc.tile_pool(name="x", bufs=1))
        # 12 bufs = 3 samples of output tiles: bufs=8 left ZERO slack
        # (eviction of block (n,kc) WAR-waits the store of block (n-2,kc),
        # and the SWDGE store stream runs at ~131 GB/s against a ~139 GB/s
        # requirement, so store lag backpressured evictions -> PSUM -> PE)
        o_pool = ctx.enter_context(tc.tile_pool(name="o", bufs=12))
        r_pool = ctx.enter_context(tc.tile_pool(name="r", bufs=2))
        mm_psum = ctx.enter_context(tc.tile_pool(name="mmp", bufs=6, space="PSUM"))
        sm_psum = ctx.enter_context(tc.tile_pool(name="smp", bufs=2, space="PSUM"))

        # ---- constants (gpsimd; cheap, before its store-only stream) ----
        nc.gpsimd.memset(warm_x[:], 0.0)
        nc.gpsimd.memset(ones_f32[:], 1.0)
        nc.vector.tensor_copy(ones[:], ones_f32[:])

        a_tiles = []
        x_tiles = []
        for n in range(NPC):
            a_tiles.append(a_pool.tile([P, KD, C], F16, name=f"a{n}", tag=f"a{n}"))
            x_tiles.append(x_pool.tile([P, KD, HW], F16, name=f"x{n}", tag=f"x{n}"))

        # ---- exp helpers ----
        e_tiles = [e_pool.tile([P, KD, C], F16, name=f"e{n}", tag=f"e{n}") for n in range(NPC)]

        def emit_exp_chunk(n, kd):
            """exp of one kd chunk of sample n (ACT, ~0.7us each)."""
            nc.scalar.activation(
                e_tiles[n][:, kd : kd + 1],
                a_tiles[n][:, kd : kd + 1],
                AF.Exp,
                bias=0.0,
                scale=1.0,
            )

        def emit_exp_slab(kd):
            """exp of sample 0's kc0 weight columns of one kd chunk (~0.2us)."""
            nc.scalar.activation(
                e_tiles[0][:, kd, 0:P], a_tiles[0][:, kd, 0:P],
                AF.Exp, bias=0.0, scale=1.0,
            )

        def emit_exp_rest(kd):
            """exp of sample 0's kc1-3 weight columns of one kd chunk (~0.5us)."""
            nc.scalar.activation(
                e_tiles[0][:, kd, P:], a_tiles[0][:, kd, P:],
                AF.Exp, bias=0.0, scale=1.0,
            )

        # ---- input loads: 2 HWDGE queues; gpsimd (SWDGE) carries no
        # loads. The SDMA engines round-robin packets across ALL in-flight
        # DMAs (receipts cluster at the end of the busy period), so the
        # only priority mechanism is limiting what is in flight: Tile
        # rotates a pool of 8 HWDGE completion semaphores and a reused
        # semaphore WAITS for its previous user, so after the 6 sample-0
        # triggers we burn the 2 remaining fresh sems on tiny decoys --
        # every bulk trigger below then only enters flight once a
        # sample-0 chunk has fully landed. ----
        # x0's kd0/kd1 quarters ride as partition-halves (2KB lines kept):
        # the round-robin is per-DMA, so 2 DMAs = double bandwidth share
        # for the earliest-needed chunks (the first matmul and the exp
        # chain gate on them; weighting kd2/kd3 instead measured +2.4us).
        # Exactly 8 sample-0 triggers use up the fresh-semaphore pool, so
        # no decoys are needed to gate the bulk loads.
        HP = P // 2
        nc.scalar.dma_start(a_tiles[0][:, 0:2], attsT[0][:, 0:2])
        nc.sync.dma_start(x_tiles[0][0:HP, 0:1], images[0][0:HP, 0:1])
        nc.sync.dma_start(x_tiles[0][HP:, 0:1], images[0][HP:, 0:1])
        nc.sync.dma_start(a_tiles[0][:, 2:4], attsT[0][:, 2:4])
        nc.scalar.dma_start(x_tiles[0][0:HP, 1:2], images[0][0:HP, 1:2])
        nc.scalar.dma_start(x_tiles[0][HP:, 1:2], images[0][HP:, 1:2])
        nc.sync.dma_start(x_tiles[0][:, 2:3], images[0][:, 2:3])
        nc.scalar.dma_start(x_tiles[0][:, 3:4], images[0][:, 3:4])
        nc.sync.dma_start(x_tiles[1][:], images[1][:])
        nc.sync.dma_start(a_tiles[1][:], attsT[1][:])
        nc.sync.dma_start(x_tiles[2][:], images[2][:])
        nc.sync.dma_start(a_tiles[2][:], attsT[2][:])
        nc.sync.dma_start(a_tiles[3][:], attsT[3][:])
        nc.sync.dma_start(x_tiles[3][:], images[3][:])

        # sample-0 exp: the four kc0-gating slabs first (0.2us each, so
        # kc0's last LDWEIGHTS unblocks ~2.7us earlier than with full
        # 0.7us kd chunks), then the kc1-3 remainders
        for kd in range(KD):
            emit_exp_slab(kd)
        for kd in range(KD):
            emit_exp_rest(kd)

        # ---- PE warm-up: NWARM dummy N=128 matmuls keep the PE busy from
        # ~7.2us with no idle window, so the HAM clock gate reaches 8/8 at
        # ~10.6us -- right before the first real matmul's data lands. Fine
        # N=128 granularity bounds the cost of leftover dummies.
        warm_ps = mm_psum.tile([P, NT], F32, name="warm_ps", tag="ps", space="PSUM")
        for _ in range(NWARM):
            nc.tensor.matmul(
                warm_ps[0:2, 0:P],
                lhsT=warm_x[:, 0:2],
                rhs=warm_x[:],
                start=True,
                stop=True,
            )

        # Stores are deferred by one sample: sample n's stores are emitted
        # during compute(n+1), after the loads have drained, alternating
        # between the SWDGE (gpsimd) queue and the by-then-idle sync queue.
        pending_stores = []

        def compute(n):
            e_t = e_tiles[n]
            x_t = x_tiles[n]
            r_sb = r_pool.tile([P, KC], F32, name=f"r{n}", tag="r")
            for kc in range(KC):
                if pending_stores:
                    dram_ap, o_prev, q = pending_stores.pop(0)
                    if q == "gpsimd":
                        nc.gpsimd.dma_start(dram_ap, o_prev)
                    else:
                        nc.sync.dma_start(dram_ap, o_prev)
                last = n == NPC - 1 and kc == KC - 1
                sm = sm_psum.tile([P, 2], F32, name=f"sm{n}_{kc}", tag="sm",
                                  space="PSUM")
                NQ = NT // 2
                if last:
                    # the final block accumulates ht1 as two N=256 groups
                    # so the last group's stop (and thus the last eviction
                    # + store) retires with a quarter-tile, shortening the
                    # last-matmul -> last-byte critical chain
                    ps0 = mm_psum.tile([P, NT], F32, name=f"ps{n}_{kc}_0",
                                       tag="ps", space="PSUM")
                    ps1a = mm_psum.tile([P, NQ], F32, name=f"ps{n}_{kc}_1a",
                                        tag="ps", space="PSUM")
                    ps1b = mm_psum.tile([P, NQ], F32, name=f"ps{n}_{kc}_1b",
                                        tag="ps", space="PSUM")
                    # matmuls grouped BY PSUM BANK (x3/e3 are long
                    # resident, order is free): the sum group stops ~1.7us
                    # before the last matmul (early reciprocal), ht0 stops
                    # ~1.1us before (its eviction + store drain overlap
                    # the ht1 matmuls), and only the small ht1b quarter's
                    # eviction -- split across ACT+DVE -- remains after
                    # the last matmul
                    def grp(ps, lo, hi):
                        for kd in range(KD):
                            nc.tensor.matmul(
                                ps[:],
                                lhsT=e_t[:, kd, kc * P : (kc + 1) * P],
                                rhs=x_t[:, kd, lo:hi] if hi > lo else ones[:],
                                start=(kd == 0),
                                stop=(kd == KD - 1),
                            )
                    grp(sm, 0, 0)
                    r_ap = r_sb[:, kc : kc + 1]
                    nc.vector.reciprocal(r_ap, sm[:, 0:1])
                    o_t = o_pool.tile([P, HW], F16, name=f"o{n}_{kc}", tag="o")
                    orow = out[n][kc * P : (kc + 1) * P]
                    grp(ps0, 0, NT)
                    nc.scalar.mul(o_t[:, 0:NQ], ps0[:, 0:NQ], r_ap)
                    nc.vector.tensor_scalar_mul(o_t[:, NQ:NT], ps0[:, NQ:], r_ap)
                    nc.sync.dma_start(orow[:, 0:NT], o_t[:, 0:NT])
                    grp(ps1a, NT, NT + NQ)
                    nc.scalar.mul(o_t[:, NT : NT + NQ], ps1a[:], r_ap)
                    grp(ps1b, NT + NQ, HW)
                    NE = NQ // 2
                    nc.scalar.mul(o_t[:, NT + NQ : NT + NQ + NE],
                                  ps1b[:, 0:NE], r_ap)
                    nc.vector.tensor_scalar_mul(o_t[:, NT + NQ + NE :],
                                                ps1b[:, NE:], r_ap)
                    # the last-ready half drains as two 64KB partition
                    # halves in parallel on BOTH HWDGE queues (a single
                    # quiet-queue 128KB store measured only ~98 GB/s --
                    # descriptor-processing bound, not bandwidth); NOT
                    # gpsimd (the SWDGE half measured 2.3us and blocked
                    # gpsimd's end-of-kernel drain)
                    nc.scalar.dma_start(orow[0:HP, NT:], o_t[0:HP, NT:])
                    nc.sync.dma_start(orow[HP:, NT:], o_t[HP:, NT:])
                    if n + 1 < NPC:
                        emit_exp_chunk(n + 1, kc)
                    continue
                ps = [
                    mm_psum.tile(
                        [P, NT], F32, name=f"ps{n}_{kc}_{ht}", tag="ps", space="PSUM"
                    )
                    for ht in range(NHT)
                ]
                for kd in range(KD):
                    lhs = e_t[:, kd, kc * P : (kc + 1) * P]
                    # tiny sum-matmul first: its kd=3 stop gates the
                    # reciprocal, so retiring it before the two 512-col
                    # matmuls shortens the eviction critical path
                    nc.tensor.matmul(
                        sm[:],
                        lhsT=lhs,
                        rhs=ones[:],
                        start=(kd == 0),
                        stop=(kd == KD - 1),
                    )
                    for ht in range(NHT):
                        nc.tensor.matmul(
                            ps[ht][:],
                            lhsT=lhs,
                            rhs=x_t[:, kd, ht * NT : (ht + 1) * NT],
                            start=(kd == 0),
                            stop=(kd == KD - 1),
                        )
                r_ap = r_sb[:, kc : kc + 1]
                nc.vector.reciprocal(r_ap, sm[:, 0:1])
                o_t = o_pool.tile([P, HW], F16, name=f"o{n}_{kc}", tag="o")
                if True:
                    # ht0 evictions: kc3's goes to DVE (an exp chunk stuck
                    # on a late DMA receipt head-of-line blocks the ACT
                    # queue, and evictions parked behind it hold PSUM banks
                    # the PE is waiting for); kc0-2's ride ACT, balancing
                    # the engines at ~5.1us each per sample (DVE carried
                    # 5.2us vs ACT 4.4us with only kc0/kc1 on ACT, and
                    # DVE's evictions gate PSUM-bank reuse)
                    if kc < 3:
                        nc.scalar.mul(o_t[:, 0:NT], ps[0][:], r_ap)
                    else:
                        nc.vector.tensor_scalar_mul(o_t[:, 0:NT], ps[0][:], r_ap)
                    nc.vector.tensor_scalar_mul(o_t[:, NT:], ps[1][:], r_ap)
                    if n == NPC - 1:
                        # last sample's early stores go on sync right
                        # after each eviction (kc2 on gpsimd fired ~2us
                        # late behind a blocked queue branch and its SWDGE
                        # drain then stretched the end-of-kernel barrier)
                        nc.sync.dma_start(out[n][kc * P : (kc + 1) * P], o_t[:])
                    else:
                        q = "sync" if n == 1 else "gpsimd"
                        pending_stores.append(
                            (out[n][kc * P : (kc + 1) * P], o_t, q)
                        )
                if n + 1 < NPC:
                    emit_exp_chunk(n + 1, kc)

        for n in range(NPC):
            compute(n)

    nc.compile()
    return nc


_NC_CACHE = None


def _get_nc():
    global _NC_CACHE
    if _NC_CACHE is None:
        _NC_CACHE = build_nc()
    return _NC_CACHE


def run(in_maps, **kwargs):
    """Run the SPMD kernel on cores 0..7. in_maps: one dict per core."""
    nc = _get_nc()
    return run_bass_kernel_spmd(nc, in_maps, core_ids=list(range(NCORES)), **kwargs)


def make_in_maps(images: np.ndarray, atts: np.ndarray):
    images = np.asarray(images, dtype=np.float32).astype(np.float16)
    atts = np.asarray(atts, dtype=np.float32)
    assert images.shape == (N, C, H, W), images.shape
    assert atts.shape == (N, C, C), atts.shape
    # x[i, n, p, kd, hw] = images[i, n, kd*128+p, hw]
    img_s = (
        images.reshape(NCORES, NPC, KD, P, HW)
        .transpose(0, 1, 3, 2, 4)
    )
    # a[i, n, p, kd, c] = atts[i*NPC+n, c, kd*128+p]  (per-sample transpose)
    attsT = (
        atts.transpose(0, 2, 1)
        .astype(np.float16)
        .reshape(NCORES, NPC, KD, P, C)
        .transpose(0, 1, 3, 2, 4)
    )
    return [
        {
            "images": np.ascontiguousarray(img_s[i]),
            "attsT": np.ascontiguousarray(attsT[i]),
        }
        for i in range(NCORES)
    ]


def kernel(images: np.ndarray, atts: np.ndarray) -> np.ndarray:
    in_maps = make_in_maps(images, atts)
    res = run(in_maps)
    outs = [res.results[i]["out"] for i in range(NCORES)]
    full = np.concatenate(outs, axis=0).reshape(N, C, H, W)
    return full.astype(np.float32)


# revision 40
# speedup vs baseline: 1.0144x; 1.0144x over previous
"""Trainium2 Bass kernel for AttentionalPlanarRemapping.

out[n,c,h,w] = sum_d softmax(atts[n,c,:])[d] * images[n,d,h,w]

Per-sample: W = softmax(atts[n]) [C,C]; out[n] = W @ images[n].reshape(C, H*W).

Sharding: data-parallel over N across 8 cores (4 samples per core).

Host preprocessing inside kernel(): both inputs are cast to fp16 and
pre-shuffled into the exact SBUF tile layout [P=128 partitions, KD, free]:
  x[n, p, kd, hw] = images[n, kd*128+p, hw]     (8KB contiguous/partition)
  a[n, p, kd, c]  = atts[n, c, kd*128+p]        (transposed: lhsT layout)
so every DMA descriptor is a 2-8KB contiguous run (the v2 trace showed atts
loads crawling at 38-110 GB/s on 1KB descriptors) and no on-device
transposition is needed. fp16 matches the PE's full-rate matmul path while
halving DMA traffic; |atts| < 6 so exp needs no max-subtraction. (fp8 was
measured offline: e4m3 on both operands gives rel_err 5.6e-2 > the 2e-2
budget, so DoubleRow is not available accuracy-wise.)

Per-core plan (v9 -- built from NTFF trace analysis over 9 iterations):
  - The early phase is chip-HBM-supply-bound: all 8 cores pull their
    sample-0 working set simultaneously, packets only start ~2.3us after
    the start barrier and the aggregate ramps 170 -> 430 GB/s, so
    sample-0's 1.5MB cannot fully land before ~14-15us no matter how the
    loads are scheduled. The design accepts that and removes every other
    stall instead.
  - The SDMA engines round-robin packets across ALL in-flight DMAs --
    per DMA, not per byte -- so (a) receipts cluster at the end of the
    busy window, (b) a chunk's bandwidth share equals its share of the
    in-flight DMA count, and (c) the only priority mechanism is limiting
    what is in flight. Sample 0 therefore rides as EXACTLY 8 triggers,
    with the earliest-needed x0 quarters (kd0, kd1) split into
    partition-halves for a double share; Tile rotates a pool of 8 HWDGE
    completion semaphores and a reused semaphore waits for its previous
    user, so every bulk trigger (the 9th onward) only enters flight once
    a sample-0 chunk has fully landed.
  - Loads ride the two HWDGE queues (sync=SP, scalar=ACT) in consumption
    order; gpsimd (SWDGE, ~131 GB/s ceiling) carries no loads. Both
    inputs are host-pre-shuffled so every descriptor is a 2-8KB
    contiguous run (1KB atts descriptors measured ~2x slower):
        scalar: a0(kd01), x0(kd1)x2, x0(kd3)
        sync:   x0(kd0)x2, a0(kd23), x0(kd2), x1, a1, x2, a2, a3, x3
    (x1..x3/a1..a3 are single full-tile triggers)
  - The PE HAM clock gate needs ~3.4us of sustained busy to reach 8/8 =
    2.4 GHz and re-throttles to 1.2 GHz after an idle window. The
    supply-paced kc0 phase (one x0 receipt per ~1us, each unlocking only
    ~0.55us of matmuls) would leave the PE ~50% idle right inside one HAM
    window -- measured as a guaranteed mid-stream re-throttle costing
    ~1.7us. So NWARM=80 dummy N=128 matmuls on a memset tile bridge the
    PE from the start barrier (~7.2us) all the way to data-ready
    (~13.4us): the real stream then runs gapless and warm to the end.
    (Cost of leftover dummies if data lands early: 53ns each.)
  - sample-0 exp runs as four 0.2us kc0-column slabs (gating kc0's
    LDWEIGHTS as early as possible -- Tile pipelines each block's
    LDWEIGHTS ahead of the previous block's matmuls, and a sem-blocked
    LDWEIGHTS head-of-line blocks the strict-FIFO PE queue) followed by
    four kc1-3 remainder chunks.
  - Main matmuls run on UNNORMALIZED E = exp(a): for each output block kc
    (128 rows of c) and each psum half ht, accumulate over kd:
        ps[ht][c128, 512] += E[kd-blk, kc-blk].T @ X[kd-blk, ht-half]
    Each [128,512] f32 psum tile is exactly one PSUM bank; 6 bufs rotate.
  - Softmax denominators ride along as tiny N=2 matmuls on the SAME loaded
    weights (sm[kc] += E.T @ ones), emitted before the two 512-col matmuls
    of each kd so the kd=3 stop retires early; r = 1/s via DVE reciprocal;
    evictions scale by r while casting to fp16 (ht0 on ACT for kc<3 else
    DVE, balancing ACT/DVE at ~5.1us each per sample; ht1 on DVE) -- at
    most one pending exp chunk can ever head-of-line block an ACT
    eviction, and DVE's evictions gate PSUM-bank reuse.
  - exp(n+1) is emitted one kd-chunk per kc block of compute(n).
  - Stores are deferred by one sample and alternate queues (n0,n2 ->
    gpsimd/SWDGE; n1 -> sync, idle once the loads drain). Sample 3 stores
    kc0/kc1 on sync and kc2 on gpsimd immediately after each eviction; the
    final kc3 block accumulates ht1 as two N=256 groups so the last
    eviction retires with a quarter tile, evicts fanned across ACT/DVE,
    and stores as two parallel halves on sync+scalar (1KB-line quarter
    stores measured only ~40 GB/s; a gpsimd half blocked SWDGE's
    end-of-kernel drain).

Measured exec_time_ns (same binary, shared devices, chip-power dependent):
v1 50-58.5us, v5 50.1us, v9 ~49.4-52us (median ~50.5); tail (last matmul
-> kernel end) 4.4us, PE stream at warm roofline throughout.
"""

import numpy as np
from contextlib import ExitStack

import concourse.bass as bass
import concourse.mybir as mybir
import concourse.tile as tile
from concourse import bacc
from concourse.bass_utils import run_bass_kernel_spmd

N, C, H, W = 32, 512, 32, 32
HW = H * W                      # 1024
NCORES = 8
NPC = N // NCORES               # 4 samples per core
P = 128
KC = C // P                     # 4 chunks over output channel c
KD = C // P                     # 4 chunks over contraction d
NT = 512                        # matmul moving free dim (one PSUM bank of f32)
NHT = HW // NT                  # 2
NWARM = 76                      # dummy N=128 matmuls bridging to data-ready
                                # (~7.2->13.4us): the supply-paced kc0 phase
                                # leaves the PE ~50% idle otherwise, which
                                # trips a HAM mid-stream re-throttle

F32 = mybir.dt.float32
F16 = mybir.dt.float16
AF = mybir.ActivationFunctionType


def build_nc():
    nc = bacc.Bacc("TRN2", target_bir_lowering=False, debug=False)

    # both inputs pre-shuffled on host into SBUF tile layout [P, KD, free]
    images = nc.dram_tensor("images", [NPC, P, KD, HW], F16, kind="ExternalInput").ap()
    attsT = nc.dram_tensor("attsT", [NPC, P, KD, C], F16, kind="ExternalInput").ap()
    out = nc.dram_tensor("out", [NPC, C, HW], F16, kind="ExternalOutput").ap()

    with ExitStack() as ctx:
        tc = ctx.enter_context(tile.TileContext(nc))

        const_pool = ctx.enter_context(tc.tile_pool(name="const", bufs=1))
        ones_f32 = const_pool.tile([P, 2], F32)
        ones = const_pool.tile([P, 2], F16)
        warm_x = const_pool.tile([P, P], F16)

        a_pool = ctx.enter_context(tc.tile_pool(name="a", bufs=1))
        e_pool = ctx.enter_context(tc.tile_pool(name="e", bufs=1))
        x_pool = ctx.enter_context(tc.tile_pool(name="x", bufs=1))
        # 12 bufs = 3 samples of output tiles: bufs=8 left ZERO slack
        # (eviction of block (n,kc) WAR-waits the store of block (n-2,kc),
        # and the SWDGE store stream runs at ~131 GB/s against a ~139 GB/s
        # requirement, so store lag backpressured evictions -> PSUM -> PE)
        o_pool = ctx.enter_context(tc.tile_pool(name="o", bufs=12))
        r_pool = ctx.enter_context(tc.tile_pool(name="r", bufs=2))
        mm_psum = ctx.enter_context(tc.tile_pool(name="mmp", bufs=6, space="PSUM"))
        sm_psum = ctx.enter_context(tc.tile_pool(name="smp", bufs=2, space="PSUM"))

        # ---- constants (gpsimd; cheap, before its store-only stream) ----
        nc.gpsimd.memset(warm_x[:], 0.0)
        nc.gpsimd.memset(ones_f32[:], 1.0)
        nc.vector.tensor_copy(ones[:], ones_f32[:])

        a_tiles = []
        x_tiles = []
        for n in range(NPC):
            a_tiles.append(a_pool.tile([P, KD, C], F16, name=f"a{n}", tag=f"a{n}"))
            x_tiles.append(x_pool.tile([P, KD, HW], F16, name=f"x{n}", tag=f"x{n}"))

        # ---- exp helpers ----
        e_tiles = [e_pool.tile([P, KD, C], F16, name=f"e{n}", tag=f"e{n}") for n in range(NPC)]

        def emit_exp_chunk(n, kd):
            """exp of one kd chunk of sample n (ACT, ~0.7us each)."""
            nc.scalar.activation(
                e_tiles[n][:, kd : kd + 1],
                a_tiles[n][:, kd : kd + 1],
                AF.Exp,
                bias=0.0,
                scale=1.0,
            )

        def emit_exp_slab(kd):
            """exp of sample 0's kc0 weight columns of one kd chunk (~0.2us)."""
            nc.scalar.activation(
                e_tiles[0][:, kd, 0:P], a_tiles[0][:, kd, 0:P],
                AF.Exp, bias=0.0, scale=1.0,
            )

        def emit_exp_rest(kd):
            """exp of sample 0's kc1-3 weight columns of one kd chunk (~0.5us)."""
            nc.scalar.activation(
                e_tiles[0][:, kd, P:], a_tiles[0][:, kd, P:],
                AF.Exp, bias=0.0, scale=1.0,
            )

        # ---- input loads: 2 HWDGE queues; gpsimd (SWDGE) carries no
        # loads. The SDMA engines round-robin packets across ALL in-flight
        # DMAs (receipts cluster at the end of the busy period), so the
        # only priority mechanism is limiting what is in flight: Tile
        # rotates a pool of 8 HWDGE completion semaphores and a reused
        # semaphore WAITS for its previous user, so after the 6 sample-0
        # triggers we burn the 2 remaining fresh sems on tiny decoys --
        # every bulk trigger below then only enters flight once a
        # sample-0 chunk has fully landed. ----
        # x0's kd0/kd1 quarters ride as partition-halves (2KB lines kept):
        # the round-robin is per-DMA, so 2 DMAs = double bandwidth share
        # for the earliest-needed chunks (the first matmul and the exp
        # chain gate on them; weighting kd2/kd3 instead measured +2.4us).
        # Exactly 8 sample-0 triggers use up the fresh-semaphore pool, so
        # no decoys are needed to gate the bulk loads.
        HP = P // 2
        nc.scalar.dma_start(a_tiles[0][:, 0:2], attsT[0][:, 0:2])
        nc.sync.dma_start(x_tiles[0][0:HP, 0:1], images[0][0:HP, 0:1])
        nc.sync.dma_start(x_tiles[0][HP:, 0:1], images[0][HP:, 0:1])
        nc.sync.dma_start(a_tiles[0][:, 2:4], attsT[0][:, 2:4])
        nc.scalar.dma_start(x_tiles[0][0:HP, 1:2], images[0][0:HP, 1:2])
        nc.scalar.dma_start(x_tiles[0][HP:, 1:2], images[0][HP:, 1:2])
        nc.sync.dma_start(x_tiles[0][:, 2:3], images[0][:, 2:3])
        nc.scalar.dma_start(x_tiles[0][:, 3:4], images[0][:, 3:4])
        nc.sync.dma_start(x_tiles[1][:], images[1][:])
        nc.sync.dma_start(a_tiles[1][:], attsT[1][:])
        nc.sync.dma_start(x_tiles[2][:], images[2][:])
        nc.sync.dma_start(a_tiles[2][:], attsT[2][:])
        nc.sync.dma_start(a_tiles[3][:], attsT[3][:])
        nc.sync.dma_start(x_tiles[3][:], images[3][:])

        # sample-0 exp: the four kc0-gating slabs first (0.2us each, so
        # kc0's last LDWEIGHTS unblocks ~2.7us earlier than with full
        # 0.7us kd chunks), then the kc1-3 remainders
        for kd in range(KD):
            emit_exp_slab(kd)
        for kd in range(KD):
            emit_exp_rest(kd)

        # ---- PE warm-up: NWARM dummy N=128 matmuls keep the PE busy from
        # ~7.2us with no idle window, so the HAM clock gate reaches 8/8 at
        # ~10.6us -- right before the first real matmul's data lands. Fine
        # N=128 granularity bounds the cost of leftover dummies.
        warm_ps = mm_psum.tile([P, NT], F32, name="warm_ps", tag="ps", space="PSUM")
        for _ in range(NWARM):
            nc.tensor.matmul(
                warm_ps[0:2, 0:P],
                lhsT=warm_x[:, 0:2],
                rhs=warm_x[:],
                start=True,
                stop=True,
            )

        # Stores are deferred by one sample: sample n's stores are emitted
        # during compute(n+1), after the loads have drained, alternating
        # between the SWDGE (gpsimd) queue and the by-then-idle sync queue.
        pending_stores = []

        def compute(n):
            e_t = e_tiles[n]
            x_t = x_tiles[n]
            r_sb = r_pool.tile([P, KC], F32, name=f"r{n}", tag="r")
            for kc in range(KC):
                if pending_stores:
                    dram_ap, o_prev, q = pending_stores.pop(0)
                    if q == "gpsimd":
                        nc.gpsimd.dma_start(dram_ap, o_prev)
                    else:
                        nc.sync.dma_start(dram_ap, o_prev)
                last = n == NPC - 1 and kc == KC - 1
                sm = sm_psum.tile([P, 2], F32, name=f"sm{n}_{kc}", tag="sm",
                                  space="PSUM")
                NQ = NT // 2
                if last:
                    # the final block accumulates ht1 as two N=256 groups
                    # so the last group's stop (and thus the last eviction
                    # + store) retires with a quarter-tile, shortening the
                    # last-matmul -> last-byte critical chain
                    ps0 = mm_psum.tile([P, NT], F32, name=f"ps{n}_{kc}_0",
                                       tag="ps", space="PSUM")
                    ps1a = mm_psum.tile([P, NQ], F32, name=f"ps{n}_{kc}_1a",
                                        tag="ps", space="PSUM")
                    ps1b = mm_psum.tile([P, NQ], F32, name=f"ps{n}_{kc}_1b",
                                        tag="ps", space="PSUM")
                    # matmuls grouped BY PSUM BANK (x3/e3 are long
                    # resident, order is free): the sum group stops ~1.7us
                    # before the last matmul (early reciprocal), ht0 stops
                    # ~1.1us before (its eviction + store drain overlap
                    # the ht1 matmuls), and only the small ht1b quarter's
                    # eviction -- split across ACT+DVE -- remains after
                    # the last matmul
                    def grp(ps, lo, hi):
                        for kd in range(KD):
                            nc.tensor.matmul(
                                ps[:],
                                lhsT=e_t[:, kd, kc * P : (kc + 1) * P],
                                rhs=x_t[:, kd, lo:hi] if hi > lo else ones[:],
                                start=(kd == 0),
                                stop=(kd == KD - 1),
                            )
                    grp(sm, 0, 0)
                    r_ap = r_sb[:, kc : kc + 1]
                    nc.vector.reciprocal(r_ap, sm[:, 0:1])
                    o_t = o_pool.tile([P, HW], F16, name=f"o{n}_{kc}", tag="o")
                    orow = out[n][kc * P : (kc + 1) * P]
                    grp(ps0, 0, NT)
                    nc.scalar.mul(o_t[:, 0:NQ], ps0[:, 0:NQ], r_ap)
                    nc.vector.tensor_scalar_mul(o_t[:, NQ:NT], ps0[:, NQ:], r_ap)
                    nc.sync.dma_start(orow[:, 0:NT], o_t[:, 0:NT])
                    grp(ps1a, NT, NT + NQ)
                    nc.scalar.mul(o_t[:, NT : NT + NQ], ps1a[:], r_ap)
                    grp(ps1b, NT + NQ, HW)
                    NE = NQ // 2
                    nc.scalar.mul(o_t[:, NT + NQ : NT + NQ + NE],
                                  ps1b[:, 0:NE], r_ap)
                    nc.vector.tensor_scalar_mul(o_t[:, NT + NQ + NE :],
                                                ps1b[:, NE:], r_ap)
                    # scalar (HWDGE) for the second half: parallel with
                    # sync's (one serialized queue measured +0.25us), and
                    # NOT gpsimd (the SWDGE half measured 2.3us transfer
                    # and blocked gpsimd's end-of-kernel drain)
                    nc.scalar.dma_start(orow[:, NT:], o_t[:, NT:])
                    if n + 1 < NPC:
                        emit_exp_chunk(n + 1, kc)
                    continue
                ps = [
                    mm_psum.tile(
                        [P, NT], F32, name=f"ps{n}_{kc}_{ht}", tag="ps", space="PSUM"
                    )
                    for ht in range(NHT)
                ]
                for kd in range(KD):
                    lhs = e_t[:, kd, kc * P : (kc + 1) * P]
                    # tiny sum-matmul first: its kd=3 stop gates the
                    # reciprocal, so retiring it before the two 512-col
                    # matmuls shortens the eviction critical path
                    nc.tensor.matmul(
                        sm[:],
                        lhsT=lhs,
                        rhs=ones[:],
                        start=(kd == 0),
                        stop=(kd == KD - 1),
                    )
                    for ht in range(NHT):
                        nc.tensor.matmul(
                            ps[ht][:],
                            lhsT=lhs,
                            rhs=x_t[:, kd, ht * NT : (ht + 1) * NT],
                            start=(kd == 0),
                            stop=(kd == KD - 1),
                        )
                r_ap = r_sb[:, kc : kc + 1]
                nc.vector.reciprocal(r_ap, sm[:, 0:1])
                o_t = o_pool.tile([P, HW], F16, name=f"o{n}_{kc}", tag="o")
                if True:
                    # ht0 evictions: kc3's goes to DVE (an exp chunk stuck
                    # on a late DMA receipt head-of-line blocks the ACT
                    # queue, and evictions parked behind it hold PSUM banks
                    # the PE is waiting for); kc0-2's ride ACT, balancing
                    # the engines at ~5.1us each per sample (DVE carried
                    # 5.2us vs ACT 4.4us with only kc0/kc1 on ACT, and
                    # DVE's evictions gate PSUM-bank reuse)
                    if kc < 3:
                        nc.scalar.mul(o_t[:, 0:NT], ps[0][:], r_ap)
                    else:
                        nc.vector.tensor_scalar_mul(o_t[:, 0:NT], ps[0][:], r_ap)
                    nc.vector.tensor_scalar_mul(o_t[:, NT:], ps[1][:], r_ap)
                    if n == NPC - 1:
                        # last sample's early stores go on sync right
                        # after each eviction (kc2 on gpsimd fired ~2us
                        # late behind a blocked queue branch and its SWDGE
                        # drain then stretched the end-of-kernel barrier)
                        nc.sync.dma_start(out[n][kc * P : (kc + 1) * P], o_t[:])
                    else:
                        q = "sync" if n == 1 else "gpsimd"
                        pending_stores.append(
                            (out[n][kc * P : (kc + 1) * P], o_t, q)
                        )
                if n + 1 < NPC:
                    emit_exp_chunk(n + 1, kc)

        for n in range(NPC):
            compute(n)

    nc.compile()
    return nc


_NC_CACHE = None


def _get_nc():
    global _NC_CACHE
    if _NC_CACHE is None:
        _NC_CACHE = build_nc()
    return _NC_CACHE


def run(in_maps, **kwargs):
    """Run the SPMD kernel on cores 0..7. in_maps: one dict per core."""
    nc = _get_nc()
    return run_bass_kernel_spmd(nc, in_maps, core_ids=list(range(NCORES)), **kwargs)


def make_in_maps(images: np.ndarray, atts: np.ndarray):
    images = np.asarray(images, dtype=np.float32).astype(np.float16)
    atts = np.asarray(atts, dtype=np.float32)
    assert images.shape == (N, C, H, W), images.shape
    assert atts.shape == (N, C, C), atts.shape
    # x[i, n, p, kd, hw] = images[i, n, kd*128+p, hw]
    img_s = (
        images.reshape(NCORES, NPC, KD, P, HW)
        .transpose(0, 1, 3, 2, 4)
    )
    # a[i, n, p, kd, c] = atts[i*NPC+n, c, kd*128+p]  (per-sample transpose)
    attsT = (
        atts.transpose(0, 2, 1)
        .astype(np.float16)
        .reshape(NCORES, NPC, KD, P, C)
        .transpose(0, 1, 3, 2, 4)
    )
    return [
        {
            "images": np.ascontiguousarray(img_s[i]),
            "attsT": np.ascontiguousarray(attsT[i]),
        }
        for i in range(NCORES)
    ]


def kernel(images: np.ndarray, atts: np.ndarray) -> np.ndarray:
    in_maps = make_in_maps(images, atts)
    res = run(in_maps)
    outs = [res.results[i]["out"] for i in range(NCORES)]
    full = np.concatenate(outs, axis=0).reshape(N, C, H, W)
    return full.astype(np.float32)


# revision 41
# speedup vs baseline: 1.0254x; 1.0109x over previous
"""Trainium2 Bass kernel for AttentionalPlanarRemapping.

out[n,c,h,w] = sum_d softmax(atts[n,c,:])[d] * images[n,d,h,w]

Per-sample: W = softmax(atts[n]) [C,C]; out[n] = W @ images[n].reshape(C, H*W).

Sharding: data-parallel over N across 8 cores (4 samples per core).

Host preprocessing inside kernel(): both inputs are cast to fp16 and
pre-shuffled into the exact SBUF tile layout [P=128 partitions, KD, free]:
  x[n, p, kd, hw] = images[n, kd*128+p, hw]     (8KB contiguous/partition)
  a[n, p, kd, c]  = atts[n, c, kd*128+p]        (transposed: lhsT layout)
so every DMA descriptor is a 2-8KB contiguous run (the v2 trace showed atts
loads crawling at 38-110 GB/s on 1KB descriptors) and no on-device
transposition is needed. fp16 matches the PE's full-rate matmul path while
halving DMA traffic; |atts| < 6 so exp needs no max-subtraction. (fp8 was
measured offline: e4m3 on both operands gives rel_err 5.6e-2 > the 2e-2
budget, so DoubleRow is not available accuracy-wise.)

Per-core plan (v9 -- built from NTFF trace analysis over 9 iterations):
  - The early phase is chip-HBM-supply-bound: all 8 cores pull their
    sample-0 working set simultaneously, packets only start ~2.3us after
    the start barrier and the aggregate ramps 170 -> 430 GB/s, so
    sample-0's 1.5MB cannot fully land before ~14-15us no matter how the
    loads are scheduled. The design accepts that and removes every other
    stall instead.
  - The SDMA engines round-robin packets across ALL in-flight DMAs --
    per DMA, not per byte -- so (a) receipts cluster at the end of the
    busy window, (b) a chunk's bandwidth share equals its share of the
    in-flight DMA count, and (c) the only priority mechanism is limiting
    what is in flight. Sample 0 therefore rides as EXACTLY 8 triggers,
    with the earliest-needed x0 quarters (kd0, kd1) split into
    partition-halves for a double share; Tile rotates a pool of 8 HWDGE
    completion semaphores and a reused semaphore waits for its previous
    user, so every bulk trigger (the 9th onward) only enters flight once
    a sample-0 chunk has fully landed.
  - Loads ride the two HWDGE queues (sync=SP, scalar=ACT) in consumption
    order; gpsimd (SWDGE, ~131 GB/s ceiling) carries no loads. Both
    inputs are host-pre-shuffled so every descriptor is a 2-8KB
    contiguous run (1KB atts descriptors measured ~2x slower):
        scalar: a0(kd01), x0(kd1)x2, x0(kd3)
        sync:   x0(kd0)x2, a0(kd23), x0(kd2), x1, a1, x2, a2, a3, x3
    (x1..x3/a1..a3 are single full-tile triggers)
  - The PE HAM clock gate needs ~3.4us of sustained busy to reach 8/8 =
    2.4 GHz and re-throttles to 1.2 GHz after an idle window. The
    supply-paced kc0 phase (one x0 receipt per ~1us, each unlocking only
    ~0.55us of matmuls) would leave the PE ~50% idle right inside one HAM
    window -- measured as a guaranteed mid-stream re-throttle costing
    ~1.7us. So NWARM=80 dummy N=128 matmuls on a memset tile bridge the
    PE from the start barrier (~7.2us) all the way to data-ready
    (~13.4us): the real stream then runs gapless and warm to the end.
    (Cost of leftover dummies if data lands early: 53ns each.)
  - sample-0 exp runs as four 0.2us kc0-column slabs (gating kc0's
    LDWEIGHTS as early as possible -- Tile pipelines each block's
    LDWEIGHTS ahead of the previous block's matmuls, and a sem-blocked
    LDWEIGHTS head-of-line blocks the strict-FIFO PE queue) followed by
    four kc1-3 remainder chunks.
  - Main matmuls run on UNNORMALIZED E = exp(a): for each output block kc
    (128 rows of c) and each psum half ht, accumulate over kd:
        ps[ht][c128, 512] += E[kd-blk, kc-blk].T @ X[kd-blk, ht-half]
    Each [128,512] f32 psum tile is exactly one PSUM bank; 6 bufs rotate.
  - Softmax denominators ride along as tiny N=2 matmuls on the SAME loaded
    weights (sm[kc] += E.T @ ones), emitted before the two 512-col matmuls
    of each kd so the kd=3 stop retires early; r = 1/s via DVE reciprocal;
    evictions scale by r while casting to fp16 (ht0 on ACT for kc<3 else
    DVE, balancing ACT/DVE at ~5.1us each per sample; ht1 on DVE) -- at
    most one pending exp chunk can ever head-of-line block an ACT
    eviction, and DVE's evictions gate PSUM-bank reuse.
  - exp(n+1) is emitted one kd-chunk per kc block of compute(n).
  - Stores are deferred by one sample and alternate queues (n0,n2 ->
    gpsimd/SWDGE; n1 -> sync, idle once the loads drain). Sample 3 stores
    kc0/kc1 on sync and kc2 on gpsimd immediately after each eviction; the
    final kc3 block accumulates ht1 as two N=256 groups so the last
    eviction retires with a quarter tile, evicts fanned across ACT/DVE,
    and stores as two parallel halves on sync+scalar (1KB-line quarter
    stores measured only ~40 GB/s; a gpsimd half blocked SWDGE's
    end-of-kernel drain).

Measured exec_time_ns (same binary, shared devices, chip-power dependent):
v1 50-58.5us, v5 50.1us, v9 ~49.4-52us (median ~50.5); tail (last matmul
-> kernel end) 4.4us, PE stream at warm roofline throughout.
"""

import numpy as np
from contextlib import ExitStack

import concourse.bass as bass
import concourse.mybir as mybir
import concourse.tile as tile
from concourse import bacc
from concourse.bass_utils import run_bass_kernel_spmd

N, C, H, W = 32, 512, 32, 32
HW = H * W                      # 1024
NCORES = 8
NPC = N // NCORES               # 4 samples per core
P = 128
KC = C // P                     # 4 chunks over output channel c
KD = C // P                     # 4 chunks over contraction d
NT = 512                        # matmul moving free dim (one PSUM bank of f32)
NHT = HW // NT                  # 2
NWARM = 76                      # dummy N=128 matmuls bridging to data-ready
                                # (~7.2->13.4us): the supply-paced kc0 phase
                                # leaves the PE ~50% idle otherwise, which
                                # trips a HAM mid-stream re-throttle

F32 = mybir.dt.float32
F16 = mybir.dt.float16
AF = mybir.ActivationFunctionType


def build_nc():
    nc = bacc.Bacc("TRN2", target_bir_lowering=False, debug=False)

    # both inputs pre-shuffled on host into SBUF tile layout [P, KD, free]
    images = nc.dram_tensor("images", [NPC, P, KD, HW], F16, kind="ExternalInput").ap()
    attsT = nc.dram_tensor("attsT", [NPC, P, KD, C], F16, kind="ExternalInput").ap()
    out = nc.dram_tensor("out", [NPC, C, HW], F16, kind="ExternalOutput").ap()

    with ExitStack() as ctx:
        tc = ctx.enter_context(tile.TileContext(nc))

        const_pool = ctx.enter_context(tc.tile_pool(name="const", bufs=1))
        ones_f32 = const_pool.tile([P, 2], F32)
        ones = const_pool.tile([P, 2], F16)
        warm_x = const_pool.tile([P, P], F16)

        a_pool = ctx.enter_context(tc.tile_pool(name="a", bufs=1))
        e_pool = ctx.enter_context(tc.tile_pool(name="e", bufs=1))
        x_pool = ctx.enter_context(tc.tile_pool(name="x", bufs=1))
        # 12 bufs = 3 samples of output tiles: bufs=8 left ZERO slack
        # (eviction of block (n,kc) WAR-waits the store of block (n-2,kc),
        # and the SWDGE store stream runs at ~131 GB/s against a ~139 GB/s
        # requirement, so store lag backpressured evictions -> PSUM -> PE)
        o_pool = ctx.enter_context(tc.tile_pool(name="o", bufs=12))
        r_pool = ctx.enter_context(tc.tile_pool(name="r", bufs=2))
        mm_psum = ctx.enter_context(tc.tile_pool(name="mmp", bufs=6, space="PSUM"))
        sm_psum = ctx.enter_context(tc.tile_pool(name="smp", bufs=2, space="PSUM"))

        # ---- constants (gpsimd; cheap, before its store-only stream) ----
        nc.gpsimd.memset(warm_x[:], 0.0)
        nc.gpsimd.memset(ones_f32[:], 1.0)
        nc.vector.tensor_copy(ones[:], ones_f32[:])

        a_tiles = []
        x_tiles = []
        for n in range(NPC):
            a_tiles.append(a_pool.tile([P, KD, C], F16, name=f"a{n}", tag=f"a{n}"))
            x_tiles.append(x_pool.tile([P, KD, HW], F16, name=f"x{n}", tag=f"x{n}"))

        # ---- exp helpers ----
        e_tiles = [e_pool.tile([P, KD, C], F16, name=f"e{n}", tag=f"e{n}") for n in range(NPC)]

        def emit_exp_chunk(n, kd):
            """exp of one kd chunk of sample n (ACT, ~0.7us each)."""
            nc.scalar.activation(
                e_tiles[n][:, kd : kd + 1],
                a_tiles[n][:, kd : kd + 1],
                AF.Exp,
                bias=0.0,
                scale=1.0,
            )

        def emit_exp_slab(kd):
            """exp of sample 0's kc0 weight columns of one kd chunk (~0.2us)."""
            nc.scalar.activation(
                e_tiles[0][:, kd, 0:P], a_tiles[0][:, kd, 0:P],
                AF.Exp, bias=0.0, scale=1.0,
            )

        def emit_exp_rest(kd):
            """exp of sample 0's kc1-3 weight columns of one kd chunk (~0.5us)."""
            nc.scalar.activation(
                e_tiles[0][:, kd, P:], a_tiles[0][:, kd, P:],
                AF.Exp, bias=0.0, scale=1.0,
            )

        # ---- input loads: 2 HWDGE queues; gpsimd (SWDGE) carries no
        # loads. The SDMA engines round-robin packets across ALL in-flight
        # DMAs (receipts cluster at the end of the busy period), so the
        # only priority mechanism is limiting what is in flight: Tile
        # rotates a pool of 8 HWDGE completion semaphores and a reused
        # semaphore WAITS for its previous user. ----
        # x0's kd0/kd1 quarters ride as partition-halves (2KB lines kept):
        # the round-robin is per-DMA, so 2 DMAs = double bandwidth share
        # for the earliest-needed chunks (the first matmul and the exp
        # chain gate on them; weighting kd2/kd3 instead measured +2.4us).
        # Exactly 8 sample-0 triggers use up the fresh-semaphore pool, so
        # no decoys are needed to gate the bulk loads.
        HP = P // 2
        nc.scalar.dma_start(a_tiles[0][:, 0:2], attsT[0][:, 0:2])
        nc.sync.dma_start(x_tiles[0][0:HP, 0:1], images[0][0:HP, 0:1])
        nc.sync.dma_start(x_tiles[0][HP:, 0:1], images[0][HP:, 0:1])
        nc.sync.dma_start(a_tiles[0][:, 2:4], attsT[0][:, 2:4])
        nc.scalar.dma_start(x_tiles[0][0:HP, 1:2], images[0][0:HP, 1:2])
        nc.scalar.dma_start(x_tiles[0][HP:, 1:2], images[0][HP:, 1:2])
        nc.sync.dma_start(x_tiles[0][:, 2:3], images[0][:, 2:3])
        nc.scalar.dma_start(x_tiles[0][:, 3:4], images[0][:, 3:4])
        nc.sync.dma_start(x_tiles[1][:], images[1][:])
        nc.sync.dma_start(a_tiles[1][:], attsT[1][:])
        nc.sync.dma_start(x_tiles[2][:], images[2][:])
        nc.sync.dma_start(a_tiles[2][:], attsT[2][:])
        nc.sync.dma_start(a_tiles[3][:], attsT[3][:])
        nc.sync.dma_start(x_tiles[3][:], images[3][:])

        # sample-0 exp: the four kc0-gating slabs first (0.2us each, so
        # kc0's last LDWEIGHTS unblocks ~2.7us earlier than with full
        # 0.7us kd chunks), then the kc1-3 remainders
        for kd in range(KD):
            emit_exp_slab(kd)
        for kd in range(KD):
            emit_exp_rest(kd)

        # ---- PE warm-up: NWARM dummy N=128 matmuls keep the PE busy from
        # ~7.2us with no idle window, so the HAM clock gate reaches 8/8 at
        # ~10.6us -- right before the first real matmul's data lands. Fine
        # N=128 granularity bounds the cost of leftover dummies.
        warm_ps = mm_psum.tile([P, NT], F32, name="warm_ps", tag="ps", space="PSUM")
        for _ in range(NWARM):
            nc.tensor.matmul(
                warm_ps[0:2, 0:P],
                lhsT=warm_x[:, 0:2],
                rhs=warm_x[:],
                start=True,
                stop=True,
            )

        # Stores are deferred by one sample: sample n's stores are emitted
        # during compute(n+1), after the loads have drained, alternating
        # between the SWDGE (gpsimd) queue and the by-then-idle sync queue.
        pending_stores = []

        def compute(n):
            e_t = e_tiles[n]
            x_t = x_tiles[n]
            r_sb = r_pool.tile([P, KC], F32, name=f"r{n}", tag="r")
            for kc in range(KC):
                if pending_stores:
                    dram_ap, o_prev, q = pending_stores.pop(0)
                    if q == "gpsimd":
                        nc.gpsimd.dma_start(dram_ap, o_prev)
                    else:
                        nc.sync.dma_start(dram_ap, o_prev)
                last = n == NPC - 1 and kc == KC - 1
                sm = sm_psum.tile([P, 2], F32, name=f"sm{n}_{kc}", tag="sm",
                                  space="PSUM")
                NQ = NT // 2
                if last:
                    # the final block accumulates ht1 as two N=256 groups
                    # so the last group's stop (and thus the last eviction
                    # + store) retires with a quarter-tile, shortening the
                    # last-matmul -> last-byte critical chain
                    ps0 = mm_psum.tile([P, NT], F32, name=f"ps{n}_{kc}_0",
                                       tag="ps", space="PSUM")
                    ps1a = mm_psum.tile([P, NQ], F32, name=f"ps{n}_{kc}_1a",
                                        tag="ps", space="PSUM")
                    ps1b = mm_psum.tile([P, NQ], F32, name=f"ps{n}_{kc}_1b",
                                        tag="ps", space="PSUM")
                    # matmuls grouped BY PSUM BANK (x3/e3 are long
                    # resident, order is free): the sum group stops ~1.7us
                    # before the last matmul (early reciprocal), ht0 stops
                    # ~1.1us before (its eviction + store drain overlap
                    # the ht1 matmuls), and only the small ht1b quarter's
                    # eviction -- split across ACT+DVE -- remains after
                    # the last matmul
                    def grp(ps, lo, hi):
                        for kd in range(KD):
                            nc.tensor.matmul(
                                ps[:],
                                lhsT=e_t[:, kd, kc * P : (kc + 1) * P],
                                rhs=x_t[:, kd, lo:hi] if hi > lo else ones[:],
                                start=(kd == 0),
                                stop=(kd == KD - 1),
                            )
                    grp(sm, 0, 0)
                    r_ap = r_sb[:, kc : kc + 1]
                    nc.vector.reciprocal(r_ap, sm[:, 0:1])
                    o_t = o_pool.tile([P, HW], F16, name=f"o{n}_{kc}", tag="o")
                    orow = out[n][kc * P : (kc + 1) * P]
                    grp(ps0, 0, NT)
                    nc.scalar.mul(o_t[:, 0:NQ], ps0[:, 0:NQ], r_ap)
                    nc.vector.tensor_scalar_mul(o_t[:, NQ:NT], ps0[:, NQ:], r_ap)
                    nc.sync.dma_start(orow[:, 0:NT], o_t[:, 0:NT])
                    grp(ps1a, NT, NT + NQ)
                    nc.scalar.mul(o_t[:, NT : NT + NQ], ps1a[:], r_ap)
                    grp(ps1b, NT + NQ, HW)
                    NE = NQ // 2
                    nc.scalar.mul(o_t[:, NT + NQ : NT + NQ + NE],
                                  ps1b[:, 0:NE], r_ap)
                    nc.vector.tensor_scalar_mul(o_t[:, NT + NQ + NE :],
                                                ps1b[:, NE:], r_ap)
                    # scalar (HWDGE) for the second half: parallel with
                    # sync's (one serialized queue measured +0.25us), and
                    # NOT gpsimd (the SWDGE half measured 2.3us transfer
                    # and blocked gpsimd's end-of-kernel drain)
                    nc.scalar.dma_start(orow[:, NT:], o_t[:, NT:])
                    if n + 1 < NPC:
                        emit_exp_chunk(n + 1, kc)
                    continue
                ps = [
                    mm_psum.tile(
                        [P, NT], F32, name=f"ps{n}_{kc}_{ht}", tag="ps", space="PSUM"
                    )
                    for ht in range(NHT)
                ]
                for kd in range(KD):
                    lhs = e_t[:, kd, kc * P : (kc + 1) * P]
                    # tiny sum-matmul first: its kd=3 stop gates the
                    # reciprocal, so retiring it before the two 512-col
                    # matmuls shortens the eviction critical path
                    nc.tensor.matmul(
                        sm[:],
                        lhsT=lhs,
                        rhs=ones[:],
                        start=(kd == 0),
                        stop=(kd == KD - 1),
                    )
                    for ht in range(NHT):
                        nc.tensor.matmul(
                            ps[ht][:],
                            lhsT=lhs,
                            rhs=x_t[:, kd, ht * NT : (ht + 1) * NT],
                            start=(kd == 0),
                            stop=(kd == KD - 1),
                        )
                r_ap = r_sb[:, kc : kc + 1]
                nc.vector.reciprocal(r_ap, sm[:, 0:1])
                o_t = o_pool.tile([P, HW], F16, name=f"o{n}_{kc}", tag="o")
                if True:
                    # ht0 evictions: kc3's goes to DVE (an exp chunk stuck
                    # on a late DMA receipt head-of-line blocks the ACT
                    # queue, and evictions parked behind it hold PSUM banks
                    # the PE is waiting for); kc0-2's ride ACT, balancing
                    # the engines at ~5.1us each per sample (DVE carried
                    # 5.2us vs ACT 4.4us with only kc0/kc1 on ACT, and
                    # DVE's evictions gate PSUM-bank reuse)
                    if kc < 3:
                        nc.scalar.mul(o_t[:, 0:NT], ps[0][:], r_ap)
                    else:
                        nc.vector.tensor_scalar_mul(o_t[:, 0:NT], ps[0][:], r_ap)
                    nc.vector.tensor_scalar_mul(o_t[:, NT:], ps[1][:], r_ap)
                    if n == NPC - 1:
                        # last sample's early stores go on sync right
                        # after each eviction (kc2 on gpsimd fired ~2us
                        # late behind a blocked queue branch and its SWDGE
                        # drain then stretched the end-of-kernel barrier)
                        nc.sync.dma_start(out[n][kc * P : (kc + 1) * P], o_t[:])
                    else:
                        q = "sync" if n == 1 else "gpsimd"
                        pending_stores.append(
                            (out[n][kc * P : (kc + 1) * P], o_t, q)
                        )
                if n + 1 < NPC:
                    emit_exp_chunk(n + 1, kc)

        for n in range(NPC):
            compute(n)

    nc.compile()
    return nc


_NC_CACHE = None


def _get_nc():
    global _NC_CACHE
    if _NC_CACHE is None:
        _NC_CACHE = build_nc()
    return _NC_CACHE


def run(in_maps, **kwargs):
    """Run the SPMD kernel on cores 0..7. in_maps: one dict per core."""
    nc = _get_nc()
    return run_bass_kernel_spmd(nc, in_maps, core_ids=list(range(NCORES)), **kwargs)


def make_in_maps(images: np.ndarray, atts: np.ndarray):
    images = np.asarray(images, dtype=np.float32).astype(np.float16)
    atts = np.asarray(atts, dtype=np.float32)
    assert images.shape == (N, C, H, W), images.shape
    assert atts.shape == (N, C, C), atts.shape
    # x[i, n, p, kd, hw] = images[i, n, kd*128+p, hw]
    img_s = (
        images.reshape(NCORES, NPC, KD, P, HW)
        .transpose(0, 1, 3, 2, 4)
    )
    # a[i, n, p, kd, c] = atts[i*NPC+n, c, kd*128+p]  (per-sample transpose)
    attsT = (
        atts.transpose(0, 2, 1)
        .astype(np.float16)
        .reshape(NCORES, NPC, KD, P, C)
        .transpose(0, 1, 3, 2, 4)
    )
    return [
        {
            "images": np.ascontiguousarray(img_s[i]),
            "attsT": np.ascontiguousarray(attsT[i]),
        }
        for i in range(NCORES)
    ]


def kernel(images: np.ndarray, atts: np.ndarray) -> np.ndarray:
    in_maps = make_in_maps(images, atts)
    res = run(in_maps)
    outs = [res.results[i]["out"] for i in range(NCORES)]
    full = np.concatenate(outs, axis=0).reshape(N, C, H, W)
    return full.astype(np.float32)
